# revision 2
# baseline (speedup 1.0000x reference)
"""Causal multi-head self-attention with RoPE on 8 Trainium2 NeuronCores.

Sharding: data-parallel over batch (4) x tensor-parallel over heads (2 groups
of 8 heads). Core c handles batch c//2, head group c%2. Each core computes a
partial output projection y_part = attnout_g @ Wo_g.T; the host sums the two
partials per batch.

Device algorithm (per core), all matmul operands bf16, f32 PSUM accumulation:
  1. QT/KT = (Wq'.T x.T) in [head_dim, seq] layout (head dims permuted to
     half-split order so RoPE pairs are partition p and p+32), V in [seq,
     head_dim] layout augmented with a ones column per head (Z accumulator).
  2. RoPE via a partition-swap DMA + cos/sin elementwise ops.
  3. Per query-chunk, TWO head-pairs' score->exp->PV chains run interleaved
     (4 independent chains hide the ACT exp latency): scoresT[j,i] blocks via
     K=64 matmuls (pair packed in PE row groups) with trapezoid column
     slicing of diagonal superblocks, exp on ACT (scale=1/8 folded in, no
     max-subtract: |scaled scores| < ~10), causal 0/1 masks on GPSIMD,
     PV + Z via M=65 matmuls accumulated over key tiles.
  4. Per chunk: Z is broadcast across each head's 64 partitions by a rank-1
     PE matmul, outT = outU * (1/Z) on DVE, and the chunk's output-projection
     matmul groups are queued and emitted interleaved into the next chunk's
     attention as PE filler work.
"""

import sys

if "/opt/trn_rl_repo" not in sys.path:
    sys.path.insert(0, "/opt/trn_rl_repo")

import numpy as np
import ml_dtypes

import concourse.bass as bass
import concourse.bacc as bacc
import concourse.tile as tile
import concourse.mybir as mybir
from concourse.bass_utils import run_bass_kernel_spmd

BF16 = ml_dtypes.bfloat16
F32 = mybir.dt.float32
BF = mybir.dt.bfloat16

B, S, D = 4, 2048, 1024
H, DK = 16, 64
HL = 8          # heads per core
NCORES = 8
THETA = 10000.0
SCALE = 1.0 / 8.0  # 1/sqrt(64)
P = 128
KT_D = D // P   # 8 k-tiles over model dim
NMT = 4         # QT/KT partition tiles (512 head dims / 128)
ST = S // P     # 16 s-tiles
IC = S // 512   # 4 query chunks of 512


def emit_program(nc, prm, reps=1):
    """Emit the whole per-core program. prm maps name -> DRAM AP."""
    xT = prm["xT"].rearrange("(kt p) s -> p kt s", p=P)        # [128, 8, 2048]
    wqT = prm["wqT"].rearrange("(kt p) o -> p kt o", p=P)      # [128, 8, 512]
    wkT = prm["wkT"].rearrange("(kt p) o -> p kt o", p=P)
    wvT = prm["wvT"].rearrange("(kt p) o -> p kt o", p=P)
    woT = prm["woT"].rearrange("(kt p) o -> p kt o", p=P)      # [128, 4, 1024]
    cosT = prm["cosT"]                                         # [128, 2048] f32
    sinT = prm["sinT"]
    maskT = prm["maskT"].rearrange("(v p) i -> p v i", p=P)    # [128, 4, 512] bf16
    y = prm["y"].rearrange("(st p) o -> p st o", p=P)          # [128, 16, 1024]

    with tile.TileContext(nc) as tc:
        import contextlib
        ctx = contextlib.ExitStack()
        with ctx:
            # --- persistent pools ---
            persist = ctx.enter_context(tc.tile_pool(name="persist", bufs=1))
            dram = ctx.enter_context(tc.tile_pool(name="dram", bufs=1, space="DRAM"))

            qrot = persist.tile([P, NMT, S], BF, tag="qrot")
            krot = persist.tile([P, NMT, S], BF, tag="krot")
            vab = persist.tile([P, ST, HL * 65], BF, tag="vab")
            masks = persist.tile([P, 4, 512], BF, tag="masks")
            wo_s = persist.tile([P, 4, D], BF, tag="wo_s")

            ones_r = persist.tile([P, P], BF, tag="ones_r")
            nc.vector.memset(ones_r[0:1, :], 1.0)

            nc.sync.dma_start(out=masks, in_=maskT)
            nc.sync.dma_start(out=wo_s, in_=woT)

            def mask_bc(v, o):
                # [128, 2, 512-o] view of masks[:, v, o:] broadcast over dim 1
                m = masks[:, v, o:512]
                return bass.AP(tensor=m.tensor, offset=m.offset,
                               ap=[m.ap[0], [0, 2], m.ap[1]])
            # ones column per head block (col 64 of each 65-wide block)
            vab_heads = vab.rearrange("p st (h c) -> p st h c", c=65)
            nc.vector.memset(vab_heads[:, :, :, 64], 1.0)

            for _ in range(reps):
                # ---------------- phase 1: projections + rope ----------------
                with tc.tile_pool(name="ph1", bufs=1) as ph1, \
                     tc.tile_pool(name="praw", bufs=3) as praw, \
                     tc.tile_pool(name="psQ", bufs=4, space="PSUM") as psA, \
                     tc.tile_pool(name="pshuf", bufs=3) as pshuf:
                    xts = ph1.tile([P, KT_D, S], BF, tag="xts")
                    wq_s = ph1.tile([P, KT_D, 512], BF, tag="wq_s")
                    wk_s = ph1.tile([P, KT_D, 512], BF, tag="wk_s")
                    wv_s = ph1.tile([P, KT_D, 512], BF, tag="wv_s")
                    cos_s = ph1.tile([P, S], F32, tag="cos_s")
                    sin_s = ph1.tile([P, S], F32, tag="sin_s")

                    for kt in range(KT_D):
                        nc.sync.dma_start(out=wq_s[:, kt, :], in_=wqT[:, kt, :])
                        nc.sync.dma_start(out=xts[:, kt, 0:512],
                                          in_=xT[:, kt, 0:512])
                    nc.sync.dma_start(out=cos_s, in_=cosT)
                    nc.sync.dma_start(out=sin_s, in_=sinT)
                    for kt in range(KT_D):
                        nc.sync.dma_start(out=wk_s[:, kt, :], in_=wkT[:, kt, :])
                        nc.sync.dma_start(out=xts[:, kt, 512:1024],
                                          in_=xT[:, kt, 512:1024])
                    for kt in range(KT_D):
                        nc.sync.dma_start(out=wv_s[:, kt, :], in_=wvT[:, kt, :])
                    for cc in range(2, 4):
                        for kt in range(KT_D):
                            nc.sync.dma_start(
                                out=xts[:, kt, cc * 512:(cc + 1) * 512],
                                in_=xT[:, kt, cc * 512:(cc + 1) * 512])

                    # Q and K projections, one 128-row (2-head) tile at a time.
                    # rot = psum*cos + shuffle(psum*(-sinT_orig)) — the
                    # partition half-swap is a SBUF->SBUF DMA; sin table is
                    # pre-negated on host so multiply happens before the swap.
                    for wt, rot in ((wq_s, qrot), (wk_s, krot)):
                        for mt in range(NMT):
                            raw = praw.tile([P, S], F32, tag="raw")
                            shuf = pshuf.tile([P, S], F32, tag="shuf")
                            for ch in range(2):
                                ps = psA.tile([P, 1024], F32, tag="psA")
                                for half in range(2):
                                    c0 = ch * 1024 + half * 512
                                    for kt in range(KT_D):
                                        nc.tensor.matmul(
                                            ps[:, half * 512:(half + 1) * 512],
                                            lhsT=wt[:, kt, mt * P:(mt + 1) * P],
                                            rhs=xts[:, kt, c0:c0 + 512],
                                            start=(kt == 0),
                                            stop=(kt == KT_D - 1),
                                        )
                                nc.vector.tensor_mul(
                                    raw[:, ch * 1024:(ch + 1) * 1024], ps, cos_s[:, ch * 1024:(ch + 1) * 1024])
                                nc.vector.tensor_mul(
                                    shuf[:, ch * 1024:(ch + 1) * 1024], ps, sin_s[:, ch * 1024:(ch + 1) * 1024])
                            shufd = pshuf.tile([P, S], F32, tag="shufd")
                            for (d0, s0) in ((0, 32), (32, 0), (64, 96), (96, 64)):
                                for ch in range(2):
                                    nc.sync.dma_start(
                                        out=shufd[d0:d0 + 32,
                                                  ch * 1024:(ch + 1) * 1024],
                                        in_=shuf[s0:s0 + 32,
                                                 ch * 1024:(ch + 1) * 1024])
                            # split the add so early query columns unblock
                            # attention before the full row is done
                            for ch in range(2):
                                nc.gpsimd.tensor_add(
                                    rot[:, mt, ch * 1024:(ch + 1) * 1024],
                                    raw[:, ch * 1024:(ch + 1) * 1024],
                                    shufd[:, ch * 1024:(ch + 1) * 1024])

                    # V projection into [seq, head_dim] with ones columns
                    for sp in range(ST // 2):
                        ps = psA.tile([P, 1024], F32, tag="psA")
                        for half in range(2):
                            st = sp * 2 + half
                            for kt in range(KT_D):
                                nc.tensor.matmul(
                                    ps[:, half * 512:(half + 1) * 512],
                                    lhsT=xts[:, kt, st * P:(st + 1) * P],
                                    rhs=wv_s[:, kt, :],
                                    start=(kt == 0),
                                    stop=(kt == KT_D - 1),
                                )
                            nc.scalar.copy(
                                out=vab_heads[:, st, :, 0:64],
                                in_=ps[:, half * 512:(half + 1) * 512].rearrange(
                                    "p (h c) -> p h c", c=64),
                            )

                # ------- phase 2: attention + per-chunk normalize/out-proj ----
                with tc.tile_pool(name="ph2", bufs=1) as ph2, \
                     tc.tile_pool(name="pexp", bufs=12) as pexp, \
                     tc.tile_pool(name="pstag", bufs=10) as pstag, \
                     tc.tile_pool(name="pzrow", bufs=10) as pzrow, \
                     tc.tile_pool(name="psA2", bufs=2, space="PSUM") as psA, \
                     tc.tile_pool(name="psPV", bufs=4, space="PSUM") as psPV, \
                     tc.tile_pool(name="pyst", bufs=4) as pyst:
                    outUT = ph2.tile([P, NMT, S], F32, tag="outUT")
                    zrep = ph2.tile([P, NMT, S], F32, tag="zrep")
                    outT = ph2.tile([P, NMT, S], BF, tag="outT")
                    stgs = {}
                    ypend = []

                    def emit_y_group():
                        if not ypend:
                            return
                        st, oc = ypend.pop(0)
                        ps = psPV.tile([P, 512], F32, tag="psPV",
                                       name=f"yps_{st}_{oc}")
                        for ht in range(NMT):
                            nc.tensor.matmul(
                                ps,
                                lhsT=outT[:, ht, st * P:(st + 1) * P],
                                rhs=wo_s[:, ht, oc * 512:(oc + 1) * 512],
                                start=(ht == 0), stop=(ht == NMT - 1),
                            )
                        ys = pyst.tile([P, 512], F32, tag="ys")
                        nc.vector.tensor_copy(out=ys, in_=ps)
                        nc.sync.dma_start(
                            out=y[:, st, oc * 512:(oc + 1) * 512], in_=ys)

                    for ic in range(IC):
                        i0 = ic * 512
                        jmax = 4 * ic + 3

                        def off(jt):
                            # trapezoid: diagonal superblock variant v
                            # only needs query cols >= 128*v
                            v = jt - 4 * ic
                            return 128 * v if v > 0 else 0

                        # two head-pairs interleaved: 4 independent
                        # score->exp->pv chains hide the exp latency
                        for hp2 in range(2):
                            hps = (2 * hp2, 2 * hp2 + 1)
                            pvs = {}
                            es = {}
                            for hp in hps:
                                for h in (2 * hp, 2 * hp + 1):
                                    pvs[h] = psPV.tile(
                                        [65, 512], F32, tag="psPV",
                                        name=f"pv_{ic}_{h}")

                            def emit_pv(hp, jt):
                                o = off(jt)
                                e = es.pop((hp, jt))
                                for idx, h in enumerate((2 * hp, 2 * hp + 1)):
                                    nc.tensor.matmul(
                                        pvs[h][:, o:512],
                                        lhsT=vab_heads[:, jt, h, :],
                                        rhs=e[:, idx, o:512],
                                        start=(jt == 0), stop=(jt == jmax),
                                    )

                            for jt in range(jmax + 1):
                                o = off(jt)
                                for hp in hps:
                                    ht = hp
                                    ps = psA.tile([P, 2, 512], F32, tag="psA",
                                                  name=f"sc_{ic}_{hp}_{jt}")
                                    for idx, h in enumerate((2 * hp, 2 * hp + 1)):
                                        po = (h % 2) * 64
                                        nc.tensor.matmul(
                                            ps[:, idx, o:512],
                                            lhsT=krot[po:po + 64, ht,
                                                      jt * P:(jt + 1) * P],
                                            rhs=qrot[po:po + 64, ht,
                                                     i0 + o:i0 + 512],
                                            start=True, stop=True,
                                        )
                                    e = pexp.tile([P, 2, 512], BF, tag="e")
                                    nc.scalar.activation(
                                        out=e[:, :, o:512], in_=ps[:, :, o:512],
                                        func=mybir.ActivationFunctionType.Exp,
                                        scale=SCALE,
                                    )
                                    v = jt - 4 * ic
                                    if v >= 0:
                                        nc.gpsimd.tensor_mul(
                                            e[:, :, o:512], e[:, :, o:512],
                                            mask_bc(v, o))
                                    es[(hp, jt)] = e
                                if jt > 0:
                                    for hp in hps:
                                        emit_pv(hp, jt - 1)
                                if jt % 2 == 1:
                                    emit_y_group()
                            for hp in hps:
                                emit_pv(hp, jmax)
                            for hp in hps:
                                ht = hp
                                for h in (2 * hp, 2 * hp + 1):
                                    po = (h % 2) * 64
                                    stg = pstag.tile([65, 512], F32, tag="stg",
                                                     name=f"stg_{ic}_{h}")
                                    nc.vector.tensor_copy(out=stg, in_=pvs[h])
                                    zrow = pzrow.tile([1, 512], BF, tag="zrow",
                                                      name=f"zrow_{ic}_{h}")
                                    nc.vector.tensor_copy(
                                        out=zrow, in_=pvs[h][64:65, :])
                                    stgs[(h, ic)] = zrow
                                    nc.sync.dma_start(
                                        out=outUT[po:po + 64, ht,
                                                  i0:i0 + 512],
                                        in_=stg[0:64, :])

                        # -- normalize this chunk: broadcast Z over the 64 --
                        # -- partitions of each head via rank-1 PE matmuls,  --
                        # -- then outT = outUT * (1/Z)                       --
                        for ht in range(NMT):
                            zb = psPV.tile([P, 512], F32, tag="psPV",
                                           name=f"zb_{ic}_{ht}")
                            for idx in range(2):
                                nc.tensor.matmul(
                                    zb[idx * 64:(idx + 1) * 64, :],
                                    lhsT=ones_r[0:1, 0:64],
                                    rhs=stgs[(2 * ht + idx, ic)],
                                    start=True, stop=True)
                            nc.vector.reciprocal(
                                out=zrep[:, ht, i0:i0 + 512], in_=zb)
                        nc.vector.tensor_mul(
                            outT[:, :, i0:i0 + 512], outUT[:, :, i0:i0 + 512],
                            zrep[:, :, i0:i0 + 512])

                        # -- queue this chunk's output projection; emitted --
                        # -- interleaved into the next chunk's attention as --
                        # -- PE filler during exp waits                     --
                        for st in range(4 * ic, 4 * ic + 4):
                            for oc in range(2):
                                ypend.append((st, oc))

                    while ypend:
                        emit_y_group()
    return nc


def build_nc(reps=1):
    nc = bacc.Bacc("TRN2", target_bir_lowering=False, debug=False,
                   num_devices=NCORES)
    prm = {}
    prm["xT"] = nc.declare_dram_parameter("xT", [D, S], BF, isOutput=False).ap()
    prm["wqT"] = nc.declare_dram_parameter("wqT", [D, 512], BF, isOutput=False).ap()
    prm["wkT"] = nc.declare_dram_parameter("wkT", [D, 512], BF, isOutput=False).ap()
    prm["wvT"] = nc.declare_dram_parameter("wvT", [D, 512], BF, isOutput=False).ap()
    prm["woT"] = nc.declare_dram_parameter("woT", [512, D], BF, isOutput=False).ap()
    prm["cosT"] = nc.declare_dram_parameter("cosT", [P, S], F32, isOutput=False).ap()
    prm["sinT"] = nc.declare_dram_parameter("sinT", [P, S], F32, isOutput=False).ap()
    prm["maskT"] = nc.declare_dram_parameter("maskT", [4 * P, 512], BF,
                                             isOutput=False).ap()
    prm["y"] = nc.declare_dram_parameter("y", [S, D], F32, isOutput=True).ap()
    emit_program(nc, prm, reps=reps)
    nc.compile()
    return nc


def host_prep(x, token_positions):
    """Shared host-side layout prep. Returns per-core input maps."""
    pos = np.asarray(token_positions).astype(np.float32)
    p = np.arange(P)
    freq = THETA ** (-np.arange(32) / 32.0)
    freqs = pos[:, None] * freq[None, :]          # [S, 32]
    cosT = np.cos(freqs)[:, p % 32].T.astype(np.float32).copy()
    # pre-shuffle sin table: the kernel computes rot = psum*cos +
    # shuffle(psum*sinT), so the sign belongs to the SOURCE partition:
    # sinT[p] = -sgn(sigma(p)) ... = +sin for p%64<32, -sin otherwise
    sgn = np.where((p % 64) < 32, 1.0, -1.0).astype(np.float32)
    sinT = (np.sin(freqs)[:, p % 32].T * sgn[:, None]).astype(np.float32).copy()

    j = np.arange(P)[:, None]
    i = np.arange(512)[None, :]
    maskT = np.concatenate(
        [(i >= j + P * v).astype(BF16) for v in range(4)], axis=0)  # [512, 512]
    return cosT, sinT, maskT


_NC_CACHE = {}


def make_in_maps(x, token_positions, Wq, Wk, Wv, Wo):
    x = np.asarray(x)
    Wq, Wk, Wv, Wo = (np.asarray(a) for a in (Wq, Wk, Wv, Wo))
    cosT, sinT, maskT = host_prep(x, token_positions)

    perm = np.empty(64, np.int64)
    perm[:32] = np.arange(32) * 2
    perm[32:] = np.arange(32) * 2 + 1

    WqT, WkT, WvT, WoT = Wq.T, Wk.T, Wv.T, Wo.T
    in_maps = []
    for c in range(NCORES):
        b, g = c // 2, c % 2
        colidx = np.concatenate([g * 512 + hl * 64 + perm for hl in range(HL)])
        in_maps.append({
            "xT": np.ascontiguousarray(x[b].T).astype(BF16),
            "wqT": np.ascontiguousarray(WqT[:, colidx]).astype(BF16),
            "wkT": np.ascontiguousarray(WkT[:, colidx]).astype(BF16),
            "wvT": np.ascontiguousarray(WvT[:, g * 512:(g + 1) * 512]).astype(BF16),
            "woT": np.ascontiguousarray(WoT[g * 512:(g + 1) * 512, :]).astype(BF16),
            "cosT": cosT,
            "sinT": sinT,
            "maskT": maskT,
        })
    return in_maps


def kernel(x, token_positions, Wq, Wk, Wv, Wo):
    in_maps = make_in_maps(x, token_positions, Wq, Wk, Wv, Wo)

    if "nc" not in _NC_CACHE:
        _NC_CACHE["nc"] = build_nc()
    nc = _NC_CACHE["nc"]

    res = run_bass_kernel_spmd(nc, in_maps, core_ids=list(range(NCORES)))
    y = np.zeros((B, S, D), np.float32)
    for c in range(NCORES):
        y[c // 2] += res.results[c]["y"]
    return y



# revision 4
# speedup vs baseline: 1.2545x; 1.2545x over previous
"""Causal multi-head self-attention with RoPE on 8 Trainium2 NeuronCores.

Sharding: data-parallel over batch (4) x tensor-parallel over heads (2 groups
of 8 heads). Core c handles batch c//2, head group c%2. Each core computes a
partial output projection y_part = attnout_g @ Wo_g.T; the host sums the two
partials per batch.

Device algorithm (per core), all matmul operands bf16, f32 PSUM accumulation:
  1. Projections blocked by 512-query chunk with the model-dim (kt) loop
     OUTER so the first PSUM group retires after one kt tile arrives; DMAs
     are issued in consumption order so PE starts ~1us in.
  2. RoPE pairs are laid out as a +-16 partition swap within each 32-block
     (host-side head-dim permutation), so the partner shuffle is a single
     DVE stream_shuffle - no SBUF-to-SBUF DMA.
  3. Attention per 512-query chunk, one head-pair per group: scoresT[j,i]
     via K=64 matmuls with trapezoid column slicing, exp on ACT (scale
     folded, no max-subtract: |scaled scores| < ~10), causal 0/1 triangle
     mask (one shared [128,128] table) multiplied on DVE over just the
     128 diagonal columns, PV + Z via M=65 matmuls accumulated over key
     tiles. PV results copy straight from PSUM into outUT via DVE
     cross-quadrant writes.
  4. Output projection groups accumulate in their own 2-bank PSUM pool and
     are emitted interleaved into the attention jt loops as PE filler.
"""

import sys

if "/opt/trn_rl_repo" not in sys.path:
    sys.path.insert(0, "/opt/trn_rl_repo")

import numpy as np
import ml_dtypes

import concourse.bass as bass
import concourse.bacc as bacc
import concourse.tile as tile
import concourse.mybir as mybir
from concourse.bass_utils import run_bass_kernel_spmd

BF16 = ml_dtypes.bfloat16
F32 = mybir.dt.float32
BF = mybir.dt.bfloat16

B, S, D = 4, 2048, 1024
H, DK = 16, 64
HL = 8          # heads per core
NCORES = 8
THETA = 10000.0
SCALE = 1.0 / 8.0  # 1/sqrt(64)
P = 128
KT_D = D // P   # 8 k-tiles over model dim
NMT = 4         # QT/KT partition tiles (512 head dims / 128)
ST = S // P     # 16 s-tiles
IC = S // 512   # 4 query chunks of 512

SHUF16 = [(i + 16) % 32 for i in range(32)]


def emit_program(nc, prm, reps=1):
    """Emit the whole per-core program. prm maps name -> DRAM AP."""
    xT = prm["xT"].rearrange("(kt p) s -> p kt s", p=P)        # [128, 8, 2048]
    wqT = prm["wqT"].rearrange("(kt p) o -> p kt o", p=P)      # [128, 8, 512]
    wkT = prm["wkT"].rearrange("(kt p) o -> p kt o", p=P)
    wvT = prm["wvT"].rearrange("(kt p) o -> p kt o", p=P)
    woT = prm["woT"].rearrange("(kt p) o -> p kt o", p=P)      # [128, 4, 1024]
    cosT = prm["cosT"]                                         # [128, 2048] f32
    sinT = prm["sinT"]
    maskT = prm["maskT"]                                       # [128, 128] bf16
    y = prm["y"].rearrange("(st p) o -> p st o", p=P)          # [128, 16, 1024]

    with tile.TileContext(nc) as tc:
        import contextlib
        ctx = contextlib.ExitStack()
        with ctx:
            # --- persistent pools (loaded once; reps model steady state) ---
            persist = ctx.enter_context(tc.tile_pool(name="persist", bufs=1))
            dram = ctx.enter_context(tc.tile_pool(name="dram", bufs=1, space="DRAM"))

            qrot = persist.tile([P, NMT, S], BF, tag="qrot")
            krot = persist.tile([P, NMT, S], BF, tag="krot")
            vab = persist.tile([P, ST, HL * 65], BF, tag="vab")
            masks = persist.tile([P, 128], BF, tag="masks")
            wo_s = persist.tile([P, 4, D], BF, tag="wo_s")
            wq_s = persist.tile([P, KT_D, 512], BF, tag="wq_s")
            wk_s = persist.tile([P, KT_D, 512], BF, tag="wk_s")
            wv_s = persist.tile([P, KT_D, 512], BF, tag="wv_s")
            cos_s = persist.tile([P, S], F32, tag="cos_s")
            sin_s = persist.tile([P, S], F32, tag="sin_s")

            ones_r = persist.tile([P, P], BF, tag="ones_r")
            nc.vector.memset(ones_r[0:1, :], 1.0)

            def mask_bc(o):
                # [128, 2, 128] view of the shared triangle, bcast over dim 1
                m = masks[:, 0:128]
                return bass.AP(tensor=m.tensor, offset=m.offset,
                               ap=[m.ap[0], [0, 2], m.ap[1]])

            # ones column per head block (col 64 of each 65-wide block)
            vab_heads = vab.rearrange("p st (h c) -> p st h c", c=65)
            nc.vector.memset(vab_heads[:, :, :, 64], 1.0)

            for rep in range(reps):
                # ---------------- phase 1: projections + rope ----------------
                with tc.tile_pool(name="ph1", bufs=1) as ph1, \
                     tc.tile_pool(name="praw", bufs=4) as praw, \
                     tc.tile_pool(name="pshuf", bufs=6) as pshuf, \
                     tc.tile_pool(name="psP", bufs=8, space="PSUM") as psP:
                    xts = ph1.tile([P, KT_D, S], BF, tag="xts")

                    # DMA issue order == consumption order: Q(qb0) paces on
                    # wq+x(qb0); K(qb0) on wk (interleaved with the cos/sin
                    # chunk 0 tables rope needs to free Q's PSUM); V(qb0) on
                    # wv; later qbs only need their x + tables.
                    for kt in range(KT_D):
                        if rep == 0:
                            nc.sync.dma_start(out=wq_s[:, kt, :],
                                              in_=wqT[:, kt, :])
                        nc.sync.dma_start(out=xts[:, kt, 0:512],
                                          in_=xT[:, kt, 0:512])
                    if rep == 0:
                        for kt in range(KT_D):
                            nc.sync.dma_start(out=wk_s[:, kt, :],
                                              in_=wkT[:, kt, :])
                            if kt == 1:
                                nc.sync.dma_start(out=cos_s[:, 0:512],
                                                  in_=cosT[:, 0:512])
                            if kt == 3:
                                nc.sync.dma_start(out=sin_s[:, 0:512],
                                                  in_=sinT[:, 0:512])
                        for kt in range(KT_D):
                            nc.sync.dma_start(out=wv_s[:, kt, :],
                                              in_=wvT[:, kt, :])
                    for cc in range(1, 4):
                        for kt in range(KT_D):
                            nc.sync.dma_start(
                                out=xts[:, kt, cc * 512:(cc + 1) * 512],
                                in_=xT[:, kt, cc * 512:(cc + 1) * 512])
                        if rep == 0:
                            nc.sync.dma_start(
                                out=cos_s[:, cc * 512:(cc + 1) * 512],
                                in_=cosT[:, cc * 512:(cc + 1) * 512])
                            nc.sync.dma_start(
                                out=sin_s[:, cc * 512:(cc + 1) * 512],
                                in_=sinT[:, cc * 512:(cc + 1) * 512])
                    if rep == 0:
                        nc.sync.dma_start(out=masks, in_=maskT)
                        nc.sync.dma_start(out=wo_s, in_=woT)

                    def rope_apply(rot, mt, c0, ps):
                        # rot = ps*cos + shuffle16(ps*sinT); sin sign folded
                        # on host so the multiply happens pre-shuffle
                        raw = praw.tile([P, 512], BF, tag="raw")
                        shuf = pshuf.tile([P, 512], BF, tag="shuf")
                        shufd = pshuf.tile([P, 512], BF, tag="shufd")
                        nc.vector.tensor_mul(shuf, ps, sin_s[:, c0:c0 + 512])
                        nc.vector.stream_shuffle(shufd, shuf, mask=SHUF16)
                        nc.vector.tensor_mul(raw, ps, cos_s[:, c0:c0 + 512])
                        nc.gpsimd.tensor_add(rot[:, mt, c0:c0 + 512],
                                             raw, shufd)

                    for qb in range(4):
                        c0 = qb * 512
                        for wt, rot in ((wq_s, qrot), (wk_s, krot)):
                            pss = [psP.tile([P, 512], F32, tag="psP",
                                            name=f"ps1_{rep}_{qb}_{id(wt)}_{mt}")
                                   for mt in range(NMT)]
                            for kt in range(KT_D):
                                for mt in range(NMT):
                                    nc.tensor.matmul(
                                        pss[mt],
                                        lhsT=wt[:, kt, mt * P:(mt + 1) * P],
                                        rhs=xts[:, kt, c0:c0 + 512],
                                        start=(kt == 0),
                                        stop=(kt == KT_D - 1),
                                    )
                            for mt in range(NMT):
                                rope_apply(rot, mt, c0, pss[mt])
                        # V projection for seq tiles of this block
                        for half in range(4):
                            st = 4 * qb + half
                            ps = psP.tile([P, 512], F32, tag="psP",
                                          name=f"psv_{rep}_{st}")
                            for kt in range(KT_D):
                                nc.tensor.matmul(
                                    ps,
                                    lhsT=xts[:, kt, st * P:(st + 1) * P],
                                    rhs=wv_s[:, kt, :],
                                    start=(kt == 0),
                                    stop=(kt == KT_D - 1),
                                )
                            nc.scalar.copy(
                                out=vab_heads[:, st, :, 0:64],
                                in_=ps.rearrange("p (h c) -> p h c", c=64),
                            )

                # ------- phase 2: attention + per-chunk normalize/out-proj ----
                with tc.tile_pool(name="ph2", bufs=1) as ph2, \
                     tc.tile_pool(name="pexp", bufs=8) as pexp, \
                     tc.tile_pool(name="pzrow", bufs=10) as pzrow, \
                     tc.tile_pool(name="precip", bufs=4) as precip, \
                     tc.tile_pool(name="psA2", bufs=2, space="PSUM") as psA, \
                     tc.tile_pool(name="psPV", bufs=2, space="PSUM") as psPV, \
                     tc.tile_pool(name="psY", bufs=2, space="PSUM") as psY, \
                     tc.tile_pool(name="pyst", bufs=4) as pyst:
                    outUT = ph2.tile([P, NMT, S], BF, tag="outUT")
                    outT = ph2.tile([P, NMT, S], BF, tag="outT")
                    zrows = {}
                    ypend = []

                    def emit_y_group():
                        if not ypend:
                            return
                        st, oc = ypend.pop(0)
                        ps = psY.tile([P, 512], F32, tag="psY",
                                      name=f"yps_{rep}_{st}_{oc}")
                        for ht in range(NMT):
                            nc.tensor.matmul(
                                ps,
                                lhsT=outT[:, ht, st * P:(st + 1) * P],
                                rhs=wo_s[:, ht, oc * 512:(oc + 1) * 512],
                                start=(ht == 0), stop=(ht == NMT - 1),
                            )
                        ys = pyst.tile([P, 512], F32, tag="ys")
                        nc.vector.tensor_copy(out=ys, in_=ps)
                        nc.sync.dma_start(
                            out=y[:, st, oc * 512:(oc + 1) * 512], in_=ys)

                    for ic in range(IC):
                        i0 = ic * 512
                        jmax = 4 * ic + 3

                        def off(jt):
                            # trapezoid: diagonal superblock variant v
                            # only needs query cols >= 128*v
                            v = jt - 4 * ic
                            return 128 * v if v > 0 else 0

                        for hp in range(4):     # one head-pair per group
                            heads = (2 * hp, 2 * hp + 1)
                            pvs = {h: psPV.tile([65, 512], F32, tag="psPV",
                                                name=f"pv_{rep}_{ic}_{h}")
                                   for h in heads}
                            es = {}

                            def emit_pv(jt):
                                o = off(jt)
                                e = es.pop(jt)
                                for idx, h in enumerate(heads):
                                    nc.tensor.matmul(
                                        pvs[h][:, o:512],
                                        lhsT=vab_heads[:, jt, h, :],
                                        rhs=e[:, idx, o:512],
                                        start=(jt == 0), stop=(jt == jmax),
                                    )

                            for jt in range(jmax + 1):
                                o = off(jt)
                                ps = psA.tile([P, 2, 512], F32, tag="psA",
                                              name=f"sc_{rep}_{ic}_{hp}_{jt}")
                                for idx, h in enumerate(heads):
                                    po = idx * 64
                                    nc.tensor.matmul(
                                        ps[:, idx, o:512],
                                        lhsT=krot[po:po + 64, hp,
                                                  jt * P:(jt + 1) * P],
                                        rhs=qrot[po:po + 64, hp,
                                                 i0 + o:i0 + 512],
                                        start=True, stop=True,
                                    )
                                e = pexp.tile([P, 2, 512], BF, tag="e")
                                nc.scalar.activation(
                                    out=e[:, :, o:512], in_=ps[:, :, o:512],
                                    func=mybir.ActivationFunctionType.Exp,
                                    scale=SCALE,
                                )
                                v = jt - 4 * ic
                                if v >= 0:
                                    nc.vector.tensor_mul(
                                        e[:, :, o:o + 128], e[:, :, o:o + 128],
                                        mask_bc(o))
                                es[jt] = e
                                if jt > 0:
                                    emit_pv(jt - 1)
                                if jt % 2 == 1:
                                    emit_y_group()
                            emit_pv(jmax)
                            # drain the pair's PSUM: outU rows straight into
                            # outUT (DVE cross-quadrant write), Z row to SBUF
                            for idx, h in enumerate(heads):
                                po = idx * 64
                                nc.vector.tensor_copy(
                                    out=outUT[po:po + 64, hp, i0:i0 + 512],
                                    in_=pvs[h][0:64, :])
                                zrow = pzrow.tile([1, 512], BF, tag="zrow",
                                                  name=f"zrow_{rep}_{ic}_{h}")
                                nc.vector.tensor_copy(
                                    out=zrow, in_=pvs[h][64:65, :])
                                zrows[(h, ic)] = zrow

                        # -- normalize this chunk: broadcast Z over the 64 --
                        # -- partitions of each head via rank-1 PE matmuls,  --
                        # -- then outT = outUT * (1/Z)                       --
                        for ht in range(NMT):
                            zb = psY.tile([P, 512], F32, tag="psY",
                                          name=f"zb_{rep}_{ic}_{ht}")
                            for idx in range(2):
                                nc.tensor.matmul(
                                    zb[idx * 64:(idx + 1) * 64, :],
                                    lhsT=ones_r[0:1, 0:64],
                                    rhs=zrows[(2 * ht + idx, ic)],
                                    start=True, stop=True)
                            recip = precip.tile([P, 512], F32, tag="recip")
                            nc.vector.reciprocal(out=recip, in_=zb)
                            nc.vector.tensor_mul(
                                outT[:, ht, i0:i0 + 512],
                                outUT[:, ht, i0:i0 + 512], recip)

                        # -- queue this chunk's output projection; emitted --
                        # -- interleaved into the next chunk's attention as --
                        # -- PE filler during exp waits                     --
                        for st in range(4 * ic, 4 * ic + 4):
                            for oc in range(2):
                                ypend.append((st, oc))

                    while ypend:
                        emit_y_group()
    return nc


def build_nc(reps=1):
    nc = bacc.Bacc("TRN2", target_bir_lowering=False, debug=False,
                   num_devices=NCORES)
    prm = {}
    prm["xT"] = nc.declare_dram_parameter("xT", [D, S], BF, isOutput=False).ap()
    prm["wqT"] = nc.declare_dram_parameter("wqT", [D, 512], BF, isOutput=False).ap()
    prm["wkT"] = nc.declare_dram_parameter("wkT", [D, 512], BF, isOutput=False).ap()
    prm["wvT"] = nc.declare_dram_parameter("wvT", [D, 512], BF, isOutput=False).ap()
    prm["woT"] = nc.declare_dram_parameter("woT", [512, D], BF, isOutput=False).ap()
    prm["cosT"] = nc.declare_dram_parameter("cosT", [P, S], F32, isOutput=False).ap()
    prm["sinT"] = nc.declare_dram_parameter("sinT", [P, S], F32, isOutput=False).ap()
    prm["maskT"] = nc.declare_dram_parameter("maskT", [P, 128], BF,
                                             isOutput=False).ap()
    prm["y"] = nc.declare_dram_parameter("y", [S, D], F32, isOutput=True).ap()
    emit_program(nc, prm, reps=reps)
    nc.compile()
    return nc


def host_prep(x, token_positions):
    """Shared host-side layout prep. Returns cos/sin/mask tables."""
    pos = np.asarray(token_positions).astype(np.float32)
    p = np.arange(P)
    pp = p % 64
    # pair index per partition: 16 pairs per 32-block, partner at +-16
    i_freq = (pp % 16) + 16 * (pp // 32)
    freq = THETA ** (-i_freq / 32.0)                  # [128]
    freqs = pos[None, :] * freq[:, None]              # [128, S]
    cosT = np.cos(freqs).astype(np.float32).copy()
    # sign belongs to the SOURCE partition of the shuffle: +sin where the
    # even element of the pair lives (j<16), -sin where the odd lives
    sgn = np.where((p % 32) < 16, 1.0, -1.0).astype(np.float32)
    sinT = (np.sin(freqs) * sgn[:, None]).astype(np.float32).copy()

    j = np.arange(P)[:, None]
    i = np.arange(128)[None, :]
    maskT = (i >= j).astype(BF16)                     # [128, 128]
    return cosT, sinT, maskT


_NC_CACHE = {}


def _perm16():
    # within-head partition -> head-dim: 32-block b holds pairs 16b..16b+15,
    # even element at j, odd at j+16 (j = partition % 32 within the block)
    q = np.empty(64, np.int64)
    q[0:16] = 2 * np.arange(16)
    q[16:32] = 2 * np.arange(16) + 1
    q[32:48] = 2 * np.arange(16, 32)
    q[48:64] = 2 * np.arange(16, 32) + 1
    return q


def make_in_maps(x, token_positions, Wq, Wk, Wv, Wo):
    x = np.asarray(x)
    Wq, Wk, Wv, Wo = (np.asarray(a) for a in (Wq, Wk, Wv, Wo))
    cosT, sinT, maskT = host_prep(x, token_positions)

    perm = _perm16()
    WqT, WkT, WvT, WoT = Wq.T, Wk.T, Wv.T, Wo.T
    in_maps = []
    for c in range(NCORES):
        b, g = c // 2, c % 2
        colidx = np.concatenate([g * 512 + hl * 64 + perm for hl in range(HL)])
        in_maps.append({
            "xT": np.ascontiguousarray(x[b].T).astype(BF16),
            "wqT": np.ascontiguousarray(WqT[:, colidx]).astype(BF16),
            "wkT": np.ascontiguousarray(WkT[:, colidx]).astype(BF16),
            "wvT": np.ascontiguousarray(WvT[:, g * 512:(g + 1) * 512]).astype(BF16),
            "woT": np.ascontiguousarray(WoT[g * 512:(g + 1) * 512, :]).astype(BF16),
            "cosT": cosT,
            "sinT": sinT,
            "maskT": maskT,
        })
    return in_maps


def kernel(x, token_positions, Wq, Wk, Wv, Wo):
    in_maps = make_in_maps(x, token_positions, Wq, Wk, Wv, Wo)

    if "nc" not in _NC_CACHE:
        _NC_CACHE["nc"] = build_nc()
    nc = _NC_CACHE["nc"]

    res = run_bass_kernel_spmd(nc, in_maps, core_ids=list(range(NCORES)))
    y = np.zeros((B, S, D), np.float32)
    for c in range(NCORES):
        y[c // 2] += res.results[c]["y"]
    return y
